# revision 9
# baseline (speedup 1.0000x reference)
"""GNN message-passing kernel for Trainium2 (8 NeuronCores, Bass/Tile).

Problem: GCNConv -> ReLU -> SAGEConv(mean) -> ReLU -> policy MLP (GELU x2) +
value head, on a 20000-node / 640000-edge random graph, 128-dim features.

Strategy (per sharding hint): shard nodes (and their output rows) across the 8
cores; partition edges by destination node. Host does *index-only* preparation
(edge bucketing by destination tile, normalization coefficients, int16 gather
indices). On device, each core:
  1. dma_gather's source-node feature rows (512B each) for its own edges,
  2. builds weighted one-hot selection matrices on the Vector engine
     (iota == dst_local) * w,
  3. segment-sums messages via TensorE matmul accumulation in PSUM:
     aggT[feat, dst] += gathered[edge, feat].T @ onehot[edge, dst],
  4. applies the dense layers as lhsT=W matmuls on the transposed activations,
     with fused bias+ReLU/GELU on the Scalar engine.
An AllGather distributes layer-1 output h (node-major) so layer 2 can gather
arbitrary source rows.  Weighted aggregation folds GCN's dinv[s]*dinv[d] and
SAGE's 1/cnt[d] into the one-hot values, so aggregation happens on raw
feat/h rows and the dense transform is applied after (linearity).
"""

import math

import numpy as np

# Problem constants (hardcoded per harness contract).
N = 20000
E = 640000
IN_DIM = 128
HID = 128
OUT = 64
NCORES = 8
P = 128  # partitions / tile width


# --------------------------------------------------------------------------
# Host-side preparation (index bookkeeping only -- all FLOPs on device)
# --------------------------------------------------------------------------

def _pack_layer(s, d, w, n, ncores):
    """Bucket edges by (core, dst-tile), pad each tile's edge list to a
    multiple of 128, and emit per-core device arrays:

      idxs [ncores, 128, 8*sum(K)] int16  -- dma_gather indices (16-wrapped,
                                             replicated to 128 partitions)
      dl   [ncores, 128, sum(K)]  f32     -- dst-local index (0..127) per edge
      vv   [ncores, 128, sum(K)]  f32     -- edge weight per edge
      K    [T] int                         -- chunks (of 128 edges) per tile,
                                             shared across cores (max)

    Edge slot i of a tile lands at partition i%128, chunk i//128 (the
    dma_gather output layout).  Pad slots use src=0, dl=0, w=0 -> they gather
    row 0 (valid data) and contribute 0 via the one-hot weight.
    """
    npn = n // ncores  # nodes per core
    t_tiles = math.ceil(npn / P)
    core = d // npn
    dloc = d % npn
    tile_id = dloc // P
    dl_val = (dloc % P).astype(np.float32)
    bucket = core * t_tiles + tile_id
    order = np.argsort(bucket, kind="stable")
    s_s = s[order]
    dl_s = dl_val[order]
    w_s = w[order]
    counts = np.bincount(bucket, minlength=ncores * t_tiles).reshape(
        ncores, t_tiles
    )
    bounds = np.zeros(ncores * t_tiles + 1, np.int64)
    np.cumsum(counts.reshape(-1), out=bounds[1:])
    kk = np.maximum((counts.max(axis=0) + P - 1) // P, 1).astype(np.int64)
    ksum = int(kk.sum())
    idxs = np.zeros((ncores, P, 8 * ksum), np.int16)
    dl = np.zeros((ncores, P, ksum), np.float32)
    vv = np.zeros((ncores, P, ksum), np.float32)
    koff = np.zeros(t_tiles + 1, np.int64)
    np.cumsum(kk, out=koff[1:])
    for c in range(ncores):
        for t in range(t_tiles):
            b = c * t_tiles + t
            lo, hi = bounds[b], bounds[b + 1]
            m = hi - lo
            slots = int(kk[t]) * P
            sp = np.zeros(slots, np.int16)
            sp[:m] = s_s[lo:hi].astype(np.int16)
            dp = np.zeros(slots, np.float32)
            dp[:m] = dl_s[lo:hi]
            wp = np.zeros(slots, np.float32)
            wp[:m] = w_s[lo:hi]
            o8 = 8 * int(koff[t])
            idxs[c, :, o8 : o8 + slots // 16] = np.tile(
                sp.reshape(slots // 16, 16).T, (8, 1)
            )
            ok = int(koff[t])
            dl[c, :, ok : ok + int(kk[t])] = dp.reshape(int(kk[t]), P).T
            vv[c, :, ok : ok + int(kk[t])] = wp.reshape(int(kk[t]), P).T
    return idxs, dl, vv, [int(x) for x in kk]


def prepare(edge_index, n, ncores):
    """All host-side index prep for both message-passing layers."""
    src = np.asarray(edge_index[0]).astype(np.int64)
    dst = np.asarray(edge_index[1]).astype(np.int64)
    indeg = np.bincount(dst, minlength=n).astype(np.float64)
    dinv = 1.0 / np.sqrt(indeg + 1.0)  # GCN adds self-loops
    sage_inv = 1.0 / np.maximum(indeg, 1.0)
    loop = np.arange(n, dtype=np.int64)
    s1 = np.concatenate([src, loop])
    d1 = np.concatenate([dst, loop])
    w1 = (dinv[s1] * dinv[d1]).astype(np.float32)
    w2 = sage_inv[dst].astype(np.float32)
    pk1 = _pack_layer(s1, d1, w1, n, ncores)
    pk2 = _pack_layer(src, dst, w2, n, ncores)
    return pk1, pk2


# --------------------------------------------------------------------------
# Device program
# --------------------------------------------------------------------------

def build_program(n, ncores, k1, k2, mlp_af=None):
    """Build the SPMD Bass/Tile program (identical on all cores; per-core
    behavior comes from per-core input data).  mlp_af overrides the MLP
    activation (CoreSim lacks Gelu; tests pass Tanh)."""
    import concourse.bass as bass  # noqa: F401
    import concourse.mybir as mybir
    import concourse.tile as tile
    from concourse import bacc

    f32 = mybir.dt.float32
    i16 = mybir.dt.int16
    AF = mybir.ActivationFunctionType
    ALU = mybir.AluOpType
    gelu_af = AF.Gelu if mlp_af is None else mlp_af

    npn = n // ncores
    t_tiles = len(k1)
    kmax = max(max(k1), max(k2))
    k1off = np.zeros(t_tiles + 1, np.int64)
    np.cumsum(k1, out=k1off[1:])
    k2off = np.zeros(t_tiles + 1, np.int64)
    np.cumsum(k2, out=k2off[1:])
    nsb = t_tiles * P  # padded node count per core in SBUF (2560)
    ngrp = math.ceil(nsb / 512)  # MLP groups of 512 columns

    nc = bacc.Bacc(
        "TRN2", target_bir_lowering=False, debug=False, num_devices=ncores
    )

    # ---- I/O ----
    feat_d = nc.dram_tensor("feat", [n, IN_DIM], f32, kind="ExternalInput")
    w_gcn_d = nc.dram_tensor("W_gcn", [IN_DIM, HID], f32, kind="ExternalInput")
    w_sl_d = nc.dram_tensor("W_sage_l", [HID, HID], f32, kind="ExternalInput")
    w_sr_d = nc.dram_tensor("W_sage_r", [HID, HID], f32, kind="ExternalInput")
    w1_d = nc.dram_tensor("W1", [HID, HID], f32, kind="ExternalInput")
    w2_d = nc.dram_tensor("W2", [HID, HID], f32, kind="ExternalInput")
    w3_d = nc.dram_tensor("W3", [HID, OUT], f32, kind="ExternalInput")
    wv_d = nc.dram_tensor("Wv", [HID, 1], f32, kind="ExternalInput")
    bias_d = nc.dram_tensor("biases", [P, 8], f32, kind="ExternalInput")
    iota_d = nc.dram_tensor("iota", [P, P], f32, kind="ExternalInput")
    ident_d = nc.dram_tensor("ident", [P, P], f32, kind="ExternalInput")
    ix1_d = nc.dram_tensor("idxs1", [P, 8 * k1off[-1]], i16, kind="ExternalInput")
    dl1_d = nc.dram_tensor("dl1", [P, int(k1off[-1])], f32, kind="ExternalInput")
    v1_d = nc.dram_tensor("v1", [P, int(k1off[-1])], f32, kind="ExternalInput")
    ix2_d = nc.dram_tensor("idxs2", [P, 8 * k2off[-1]], i16, kind="ExternalInput")
    dl2_d = nc.dram_tensor("dl2", [P, int(k2off[-1])], f32, kind="ExternalInput")
    v2_d = nc.dram_tensor("v2", [P, int(k2off[-1])], f32, kind="ExternalInput")
    means_d = nc.dram_tensor("meansT_out", [OUT, npn], f32, kind="ExternalOutput")
    vals_d = nc.dram_tensor("vals_out", [1, npn], f32, kind="ExternalOutput")

    with tile.TileContext(nc) as tc:
        with (
            tc.tile_pool(name="const", bufs=1) as cp,
            tc.tile_pool(name="gath", bufs=3) as gp,
            tc.tile_pool(name="oh", bufs=6) as ohp,
            tc.tile_pool(name="sbm", bufs=3) as sbm,
            tc.tile_pool(name="mlp", bufs=2) as mlpp,
            tc.tile_pool(name="ps_agg", bufs=2, space="PSUM") as pag,
            tc.tile_pool(name="ps_small", bufs=2, space="PSUM") as psm,
            tc.tile_pool(name="ps_mlp", bufs=2, space="PSUM") as pml,
            tc.tile_pool(name="ps_out", bufs=1, space="PSUM") as pou,
            tc.tile_pool(name="dram", bufs=1, space="DRAM") as dp,
        ):
            # ---- constants into SBUF ----
            wg_sb = cp.tile_from(w_gcn_d[:, :])
            wsl_sb = cp.tile_from(w_sl_d[:, :])
            wsr_sb = cp.tile_from(w_sr_d[:, :])
            w1_sb = cp.tile_from(w1_d[:, :])
            w2_sb = cp.tile_from(w2_d[:, :])
            w3_sb = cp.tile_from(w3_d[:, :])
            wv_sb = cp.tile_from(wv_d[:, :])
            bias_sb = cp.tile_from(bias_d[:, :])
            iota_sb = cp.tile_from(iota_d[:, :])
            ident_sb = cp.tile_from(ident_d[:, :])
            ix1_sb = cp.tile_from(ix1_d[:, :])
            dl1_sb = cp.tile_from(dl1_d[:, :])
            v1_sb = cp.tile_from(v1_d[:, :])
            ix2_sb = cp.tile_from(ix2_d[:, :])
            dl2_sb = cp.tile_from(dl2_d[:, :])
            v2_sb = cp.tile_from(v2_d[:, :])
            # biases: column b of bias_d = [b_gcn, b_sage_l, b1, b2, b3, bv]
            b_gcn = bias_sb[:, 0:1]
            b_sage = bias_sb[:, 1:2]
            b1c = bias_sb[:, 2:3]
            b2c = bias_sb[:, 3:4]
            b3c = bias_sb[:, 4:5]
            bvc = bias_sb[:, 5:6]

            h_sb = cp.tile([P, nsb], f32)     # layer-1 out, [hid, node] layout
            h2_sb = cp.tile([P, nsb], f32)    # layer-2 out
            means_sb = cp.tile([OUT, nsb], f32)
            vals_sb = cp.tile([1, nsb], f32)

            h_own = dp.tile([npn, HID], f32)
            h_full = dp.tile([n, HID], f32, addr_space="Shared")

            def mp_layer(lname, src_dram, koff, klist, dl_sb, v_sb, ix_sb):
                """One message-passing aggregation layer: returns per-tile
                PSUM->SBUF aggregated tiles via callback."""
                agg_tiles = []
                gsplit = 8  # chunks per dma_gather: 1024 idxs = 65 descs/lane
                # (HW SWDGE ring capacity limit is between 65 and 81)
                for t in range(t_tiles):
                    kt = klist[t]
                    g = gp.tile([P, kmax, P], f32, name=f"g_{lname}_{t}", tag="g")
                    for j in range(0, kt, gsplit):
                        jc = min(gsplit, kt - j)
                        o8 = 8 * (int(koff[t]) + j)
                        nc.gpsimd.dma_gather(
                            g[:, j : j + jc, :],
                            src_dram[:, :],
                            ix_sb[:, o8 : o8 + 8 * jc],
                            jc * P,
                            jc * P,
                            P,
                        )
                    ps = pag.tile([P, P], f32, name=f"agg_{lname}_{t}", tag="agg")
                    for k in range(kt):
                        c = int(koff[t]) + k
                        oh = ohp.tile([P, P], f32, name=f"oh_{lname}_{t}_{k}", tag="oh")
                        nc.vector.tensor_scalar(
                            oh[:, :],
                            iota_sb[:, :],
                            dl_sb[:, c : c + 1],
                            v_sb[:, c : c + 1],
                            ALU.is_equal,
                            ALU.mult,
                        )
                        nc.tensor.matmul(
                            ps[:, :],
                            g[:, k, :],
                            oh[:, :],
                            start=(k == 0),
                            stop=(k == kt - 1),
                        )
                    agg = sbm.tile([P, P], f32, name=f"aggsb_{lname}_{t}", tag="aggsb")
                    nc.vector.tensor_copy(agg[:, :], ps[:, :])
                    agg_tiles.append(agg)
                return agg_tiles

            # ======== Layer 1: GCN ========
            agg1 = mp_layer("l1", feat_d, k1off, k1, dl1_sb, v1_sb, ix1_sb)
            for t in range(t_tiles):
                ph = psm.tile([P, P], f32, name=f"ph_{t}", tag="ps_s")
                nc.tensor.matmul(
                    ph[:, :], wg_sb[:, :], agg1[t][:, :], start=True, stop=True
                )
                nc.scalar.activation(
                    h_sb[:, t * P : (t + 1) * P], ph[:, :], AF.Relu, bias=b_gcn
                )
                # transpose to node-major and store for AllGather
                pt = psm.tile([P, P], f32, name=f"pt_{t}", tag="ps_s")
                nc.tensor.transpose(
                    pt[:, :], h_sb[:, t * P : (t + 1) * P], ident_sb[:, :]
                )
                hnm = sbm.tile([P, P], f32, name=f"hnm_{t}", tag="hnm")
                nc.vector.tensor_copy(hnm[:, :], pt[:, :])
                rows = min(P, npn - t * P)
                nc.sync.dma_start(
                    h_own[t * P : t * P + rows, :], hnm[:rows, :]
                )

            # ======== AllGather h ========
            nc.gpsimd.collective_compute(
                "AllGather",
                mybir.AluOpType.bypass,
                replica_groups=[list(range(ncores))],
                ins=[h_own.opt()],
                outs=[h_full.opt()],
            )

            # ======== Layer 2: SAGE(mean) ========
            agg2 = mp_layer("l2", h_full, k2off, k2, dl2_sb, v2_sb, ix2_sb)
            for t in range(t_tiles):
                pz = psm.tile([P, P], f32, name=f"pz_{t}", tag="ps_s")
                nc.tensor.matmul(
                    pz[:, :], wsl_sb[:, :], agg2[t][:, :], start=True, stop=False
                )
                nc.tensor.matmul(
                    pz[:, :],
                    wsr_sb[:, :],
                    h_sb[:, t * P : (t + 1) * P],
                    start=False,
                    stop=True,
                )
                nc.scalar.activation(
                    h2_sb[:, t * P : (t + 1) * P], pz[:, :], AF.Relu, bias=b_sage
                )

            # ======== Policy MLP + value head (groups of 512 cols) ========
            for g_i in range(ngrp):
                lo = g_i * 512
                hi = min(nsb, lo + 512)
                w = hi - lo
                p1 = pml.tile([P, 512], f32, name=f"p1_{g_i}", tag="pmlp")[:, :w]
                nc.tensor.matmul(
                    p1, w1_sb[:, :], h2_sb[:, lo:hi], start=True, stop=True
                )
                z1 = mlpp.tile([P, 512], f32, name=f"z1_{g_i}", tag="z")[:, :w]
                nc.scalar.activation(z1, p1, gelu_af, bias=b1c)
                p2 = pml.tile([P, 512], f32, name=f"p2_{g_i}", tag="pmlp")[:, :w]
                nc.tensor.matmul(p2, w2_sb[:, :], z1, start=True, stop=True)
                z2 = mlpp.tile([P, 512], f32, name=f"z2_{g_i}", tag="z")[:, :w]
                nc.scalar.activation(z2, p2, gelu_af, bias=b2c)
                p3 = pou.tile([OUT, 512], f32, name=f"p3_{g_i}", tag="pout")[:, :w]
                nc.tensor.matmul(p3, w3_sb[:, :], z2, start=True, stop=True)
                nc.vector.tensor_scalar_add(means_sb[:, lo:hi], p3, b3c[:OUT, :])
                pv = pou.tile([1, 512], f32, name=f"pv_{g_i}", tag="pv")[:, :w]
                nc.tensor.matmul(pv, wv_sb[:, :], h2_sb[:, lo:hi], start=True, stop=True)
                nc.vector.tensor_scalar_add(vals_sb[:, lo:hi], pv, bvc[:1, :])

            # ======== outputs ========
            nc.sync.dma_start(means_d[:, :], means_sb[:, :npn])
            nc.sync.dma_start(vals_d[:, :], vals_sb[:, :npn])

    nc.compile()
    return nc


# --------------------------------------------------------------------------
# In-map assembly + entry point
# --------------------------------------------------------------------------

def make_in_maps(inputs, pk1, pk2, n, ncores):
    f = np.ascontiguousarray(np.asarray(inputs["feat"], np.float32))
    ix1, dl1, v1, _ = pk1
    ix2, dl2, v2, _ = pk2
    biases = np.zeros((P, 8), np.float32)
    for col, key in enumerate(["b_gcn", "b_sage_l", "b1", "b2", "b3", "bv"]):
        b = np.asarray(inputs[key], np.float32).reshape(-1)
        biases[: b.shape[0], col] = b
    iota = np.tile(np.arange(P, dtype=np.float32), (P, 1))
    ident = np.eye(P, dtype=np.float32)
    common = {
        "feat": f,
        "W_gcn": np.ascontiguousarray(np.asarray(inputs["W_gcn"], np.float32)),
        "W_sage_l": np.ascontiguousarray(np.asarray(inputs["W_sage_l"], np.float32)),
        "W_sage_r": np.ascontiguousarray(np.asarray(inputs["W_sage_r"], np.float32)),
        "W1": np.ascontiguousarray(np.asarray(inputs["W1"], np.float32)),
        "W2": np.ascontiguousarray(np.asarray(inputs["W2"], np.float32)),
        "W3": np.ascontiguousarray(np.asarray(inputs["W3"], np.float32)),
        "Wv": np.ascontiguousarray(np.asarray(inputs["Wv"], np.float32)),
        "biases": biases,
        "iota": iota,
        "ident": ident,
    }
    in_maps = []
    for c in range(ncores):
        m = dict(common)
        m["idxs1"] = np.ascontiguousarray(ix1[c])
        m["dl1"] = np.ascontiguousarray(dl1[c])
        m["v1"] = np.ascontiguousarray(v1[c])
        m["idxs2"] = np.ascontiguousarray(ix2[c])
        m["dl2"] = np.ascontiguousarray(dl2[c])
        m["v2"] = np.ascontiguousarray(v2[c])
        in_maps.append(m)
    return in_maps


_CACHE = {}


def _get_program(n, ncores, k1, k2):
    key = (n, ncores, tuple(k1), tuple(k2))
    if key not in _CACHE:
        _CACHE[key] = build_program(n, ncores, k1, k2)
    return _CACHE[key]


def _ensure_ntff_hook():
    """This image's ``antenv`` lacks ``axon_hooks``; register a shim module
    and wire the ctypes NTFF profile hook from the boot helper so
    ``run_bass_kernel_spmd(trace=True)`` can capture profiles."""
    import sys
    import types

    try:
        import antenv.axon_hooks  # noqa: F401

        return
    except ImportError:
        pass
    import antenv

    mod = types.ModuleType("antenv.axon_hooks")
    state = {"hook": None}
    mod.set_axon_ntff_profile_hook = lambda h: state.__setitem__("hook", h)
    mod.get_axon_ntff_profile_hook = lambda: state["hook"]
    sys.modules["antenv.axon_hooks"] = mod
    antenv.axon_hooks = mod
    try:
        from trn_agent_boot.trn_boot import _ntff_profile_via_ctypes

        so = "/opt/axon/libaxon_pjrt.so"
        import os

        if os.path.exists(so):
            state["hook"] = _ntff_profile_via_ctypes(so)
    except Exception:
        pass


def run(inputs, n=N, ncores=NCORES, trace=False):
    """Full pipeline: prep, build/compile (cached), execute, unshard."""
    from concourse.bass_utils import run_bass_kernel_spmd

    if trace:
        _ensure_ntff_hook()

    pk1, pk2 = prepare(np.asarray(inputs["edge_index"]), n, ncores)
    nc = _get_program(n, ncores, pk1[3], pk2[3])
    in_maps = make_in_maps(inputs, pk1, pk2, n, ncores)
    res = run_bass_kernel_spmd(
        nc, in_maps, core_ids=list(range(ncores)), trace=trace
    )
    npn = n // ncores
    means = np.empty((n, OUT), np.float32)
    values = np.empty((n,), np.float32)
    for c in range(ncores):
        means[c * npn : (c + 1) * npn, :] = res.results[c]["meansT_out"].T
        values[c * npn : (c + 1) * npn] = res.results[c]["vals_out"][0]
    return (means, values), res


def kernel(**inputs):
    out, _ = run(inputs)
    return out


# revision 13
# speedup vs baseline: 1.6919x; 1.6919x over previous
"""GNN message-passing kernel for Trainium2 (8 NeuronCores, Bass/Tile).

Problem: GCNConv -> ReLU -> SAGEConv(mean) -> ReLU -> policy MLP (GELU x2) +
value head, on a 20000-node / 640000-edge random graph, 128-dim features.

Strategy (per sharding hint): shard nodes/output rows across the 8 cores;
partition edges by destination node.  Host does *index-only* preparation
(edge bucketing by destination tile, int16 gather indices, normalization
coefficient tables).  Device pipeline per core:

  phase 0   feat_scaled[n] = feat[n] * dinv[n]  (bf16 table in DRAM)
  layer 1   per dst tile: dma_gather feat_scaled rows for the tile's edges
            (1024-idx pieces round-robined over 4 SWDGE queues -- the
            descriptor ring is tiny and a single queue serializes), build
            0/1 one-hot selection matrices on VectorE (iota == dst_local,
            one bf16 tensor_scalar per 128-edge chunk), segment-sum via
            TensorE matmul accumulation: agg[d,f] += onehot[e,d].T @ g[e,f].
            Scale by dinv[d] (per-partition) while copying PSUM->SBUF, PE-
            transpose to [f,d], then h = relu(W_gcn.T @ aggT + b).
            GCN self-loops ride along as ordinary edges: gathered value is
            dinv[d]*feat[d] and the output-side dinv[d] completes dinv^2.
  allgather h (node-major bf16) so layer 2 can gather any source row.
  layer 2   same aggregation over h with scale 1/max(cnt[d],1) (SAGE mean),
            then h2 = relu(W_l.T @ meanT + b_l + W_r.T @ hT).
  mlp       z=gelu(W1.T z+b1) x2, means=W3.T z+b3, values=Wv.T h2+bv in
            512-column batches.  Outputs stored transposed; host re-stacks.

Aggregation runs in bf16 (fp32 PSUM accumulate); rounding errors are
independent per edge so the segment means stay ~1e-3 accurate.
"""

import math

import numpy as np

# Problem constants (hardcoded per harness contract).
N = 20000
E = 640000
IN_DIM = 128
HID = 128
OUT = 64
NCORES = 8
P = 128  # partitions / tile width
GSPLIT = 8  # chunks per dma_gather: 1024 idxs = 65 descs/lane (ring limit ~80)
NQUEUES = 4


# --------------------------------------------------------------------------
# Host-side preparation (index bookkeeping only -- all FLOPs on device)
# --------------------------------------------------------------------------

def _pack_layer(s, d, n, ncores):
    """Bucket edges by (core, dst-tile), pad each tile's edge list to a
    multiple of 128, and emit per-core device arrays:

      idxs [ncores, 128, 8*sum(K)] int16  -- dma_gather indices (16-wrapped,
                                             replicated to 128 partitions)
      dl   [ncores, 128, sum(K)]  f32     -- dst-local index (0..127) / edge
      K    [T] int                         -- chunks (of 128 edges) per tile

    Edge slot i of a tile lands at partition i%128, chunk i//128 (dma_gather
    output layout).  Pad slots: src=0, dl=-1 (one-hot row all zero).
    """
    npn = n // ncores
    t_tiles = math.ceil(npn / P)
    core = d // npn
    dloc = d % npn
    tile_id = dloc // P
    dl_val = (dloc % P).astype(np.float32)
    bucket = core * t_tiles + tile_id
    order = np.argsort(bucket, kind="stable")
    s_s = s[order]
    dl_s = dl_val[order]
    counts = np.bincount(bucket, minlength=ncores * t_tiles).reshape(
        ncores, t_tiles
    )
    bounds = np.zeros(ncores * t_tiles + 1, np.int64)
    np.cumsum(counts.reshape(-1), out=bounds[1:])
    kk = np.maximum((counts.max(axis=0) + P - 1) // P, 1).astype(np.int64)
    ksum = int(kk.sum())
    idxs = np.zeros((ncores, P, 8 * ksum), np.int16)
    dl = np.full((ncores, P, ksum), -1.0, np.float32)
    koff = np.zeros(t_tiles + 1, np.int64)
    np.cumsum(kk, out=koff[1:])
    for c in range(ncores):
        for t in range(t_tiles):
            b = c * t_tiles + t
            lo, hi = bounds[b], bounds[b + 1]
            m = hi - lo
            slots = int(kk[t]) * P
            sp = np.zeros(slots, np.int16)
            sp[:m] = s_s[lo:hi].astype(np.int16)
            dp = np.full(slots, -1.0, np.float32)
            dp[:m] = dl_s[lo:hi]
            o8 = 8 * int(koff[t])
            idxs[c, :, o8 : o8 + slots // 16] = np.tile(
                sp.reshape(slots // 16, 16).T, (8, 1)
            )
            ok = int(koff[t])
            dl[c, :, ok : ok + int(kk[t])] = dp.reshape(int(kk[t]), P).T
    return idxs, dl, [int(x) for x in kk]


def prepare(edge_index, n, ncores):
    """All host-side index prep for both message-passing layers."""
    src = np.asarray(edge_index[0]).astype(np.int64)
    dst = np.asarray(edge_index[1]).astype(np.int64)
    npn = n // ncores
    t_tiles = math.ceil(npn / P)
    indeg = np.bincount(dst, minlength=n).astype(np.float64)
    dinv = (1.0 / np.sqrt(indeg + 1.0)).astype(np.float32)  # GCN w/ self-loop
    cntinv = (1.0 / np.maximum(indeg, 1.0)).astype(np.float32)
    loop = np.arange(n, dtype=np.int64)
    s1 = np.concatenate([src, loop])
    d1 = np.concatenate([dst, loop])
    pk1 = _pack_layer(s1, d1, n, ncores)
    pk2 = _pack_layer(src, dst, n, ncores)
    # per-node tables, wrapped [128, n/128]: node m at [m%128, m//128]
    nstr = math.ceil(n / P)
    dpad = np.concatenate([dinv, np.zeros(nstr * P - n, np.float32)])
    dinv_all = np.ascontiguousarray(dpad.reshape(nstr, P).T)
    # per-core output-side scales per own tile: [128, T]
    sc1 = np.zeros((ncores, P, t_tiles), np.float32)
    sc2 = np.zeros((ncores, P, t_tiles), np.float32)
    for c in range(ncores):
        own = np.arange(c * npn, (c + 1) * npn)
        pad = t_tiles * P - npn
        d1v = np.concatenate([dinv[own], np.zeros(pad, np.float32)])
        d2v = np.concatenate([cntinv[own], np.zeros(pad, np.float32)])
        sc1[c] = d1v.reshape(t_tiles, P).T
        sc2[c] = d2v.reshape(t_tiles, P).T
    return pk1, pk2, dinv_all, sc1, sc2


# --------------------------------------------------------------------------
# Device program
# --------------------------------------------------------------------------

def build_program(n, ncores, k1, k2, mlp_af=None):
    """Build the SPMD Bass/Tile program (identical on all cores; per-core
    behavior comes from per-core input data).  mlp_af overrides the MLP
    activation (CoreSim lacks Gelu; tests pass Tanh)."""
    import concourse.bass as bass  # noqa: F401
    import concourse.mybir as mybir
    import concourse.tile as tile
    from concourse import bacc

    f32 = mybir.dt.float32
    bf16 = mybir.dt.bfloat16
    i16 = mybir.dt.int16
    AF = mybir.ActivationFunctionType
    ALU = mybir.AluOpType
    gelu_af = AF.Gelu if mlp_af is None else mlp_af

    npn = n // ncores
    t_tiles = len(k1)
    kmax = max(max(k1), max(k2))
    nstr = math.ceil(n / P)
    k1off = np.zeros(t_tiles + 1, np.int64)
    np.cumsum(k1, out=k1off[1:])
    k2off = np.zeros(t_tiles + 1, np.int64)
    np.cumsum(k2, out=k2off[1:])
    nsb = t_tiles * P  # padded node count per core in SBUF (2560)
    ngrp = math.ceil(nsb / 512)  # MLP groups of 512 columns

    nc = bacc.Bacc(
        "TRN2",
        target_bir_lowering=False,
        debug=False,
        num_devices=ncores,
        num_swdge_queues=NQUEUES,
    )
    qc = [0]  # SWDGE queue rotation counter

    # ---- I/O ----
    feat_d = nc.dram_tensor("feat", [n, IN_DIM], f32, kind="ExternalInput")
    w_gcn_d = nc.dram_tensor("W_gcn", [IN_DIM, HID], bf16, kind="ExternalInput")
    w_sl_d = nc.dram_tensor("W_sage_l", [HID, HID], bf16, kind="ExternalInput")
    w_sr_d = nc.dram_tensor("W_sage_r", [HID, HID], bf16, kind="ExternalInput")
    w1_d = nc.dram_tensor("W1", [HID, HID], bf16, kind="ExternalInput")
    w2_d = nc.dram_tensor("W2", [HID, HID], bf16, kind="ExternalInput")
    w3_d = nc.dram_tensor("W3", [HID, OUT], bf16, kind="ExternalInput")
    wv_d = nc.dram_tensor("Wv", [HID, 1], bf16, kind="ExternalInput")
    bias_d = nc.dram_tensor("biases", [P, 8], f32, kind="ExternalInput")
    iota_d = nc.dram_tensor("iota", [P, P], bf16, kind="ExternalInput")
    ident_d = nc.dram_tensor("ident", [P, P], bf16, kind="ExternalInput")
    dinv_d = nc.dram_tensor("dinv_all", [P, nstr], f32, kind="ExternalInput")
    sc1_d = nc.dram_tensor("sc1", [P, t_tiles], f32, kind="ExternalInput")
    sc2_d = nc.dram_tensor("sc2", [P, t_tiles], f32, kind="ExternalInput")
    ix1_d = nc.dram_tensor("idxs1", [P, 8 * k1off[-1]], i16, kind="ExternalInput")
    dl1_d = nc.dram_tensor("dl1", [P, int(k1off[-1])], f32, kind="ExternalInput")
    ix2_d = nc.dram_tensor("idxs2", [P, 8 * k2off[-1]], i16, kind="ExternalInput")
    dl2_d = nc.dram_tensor("dl2", [P, int(k2off[-1])], f32, kind="ExternalInput")
    means_d = nc.dram_tensor("meansT_out", [OUT, npn], f32, kind="ExternalOutput")
    vals_d = nc.dram_tensor("vals_out", [1, npn], f32, kind="ExternalOutput")

    with tile.TileContext(nc) as tc:
        with (
            tc.tile_pool(name="const", bufs=1) as cp,
            tc.tile_pool(name="fsc", bufs=4) as fscp,
            tc.tile_pool(name="gath", bufs=3) as gp,
            tc.tile_pool(name="oh", bufs=8) as ohp,
            tc.tile_pool(name="sbm", bufs=3) as sbm,
            tc.tile_pool(name="mlp", bufs=2) as mlpp,
            tc.tile_pool(name="ps_agg", bufs=2, space="PSUM") as pag,
            tc.tile_pool(name="ps_t", bufs=2, space="PSUM") as pst,
            tc.tile_pool(name="ps_h", bufs=1, space="PSUM") as psh,
            tc.tile_pool(name="ps_mlp", bufs=1, space="PSUM") as pml,
            tc.tile_pool(name="ps_out", bufs=1, space="PSUM") as pou,
            tc.tile_pool(name="dram", bufs=1, space="DRAM") as dp,
        ):
            # ---- constants into SBUF ----
            wg_sb = cp.tile_from(w_gcn_d[:, :])
            wsl_sb = cp.tile_from(w_sl_d[:, :])
            wsr_sb = cp.tile_from(w_sr_d[:, :])
            w1_sb = cp.tile_from(w1_d[:, :])
            w2_sb = cp.tile_from(w2_d[:, :])
            w3_sb = cp.tile_from(w3_d[:, :])
            wv_sb = cp.tile_from(wv_d[:, :])
            bias_sb = cp.tile_from(bias_d[:, :])
            iota_sb = cp.tile_from(iota_d[:, :])
            ident_sb = cp.tile_from(ident_d[:, :])
            dinv_sb = cp.tile_from(dinv_d[:, :])
            sc1_sb = cp.tile_from(sc1_d[:, :])
            sc2_sb = cp.tile_from(sc2_d[:, :])
            ix1_sb = cp.tile_from(ix1_d[:, :])
            dl1_sb = cp.tile_from(dl1_d[:, :])
            ix2_sb = cp.tile_from(ix2_d[:, :])
            dl2_sb = cp.tile_from(dl2_d[:, :])
            b_gcn = bias_sb[:, 0:1]
            b_sage = bias_sb[:, 1:2]
            b1c = bias_sb[:, 2:3]
            b2c = bias_sb[:, 3:4]
            b3c = bias_sb[:, 4:5]
            bvc = bias_sb[:, 5:6]

            h_sb = cp.tile([P, nsb], bf16)    # layer-1 out, [hid, node]
            h2_sb = cp.tile([P, nsb], bf16)   # layer-2 out
            means_sb = cp.tile([OUT, nsb], f32)
            vals_sb = cp.tile([1, nsb], f32)

            feat_sc = dp.tile([nstr * P, IN_DIM], bf16)
            h_own = dp.tile([npn, HID], bf16)
            h_full = dp.tile([n, HID], bf16, addr_space="Shared")

            # ---- phase 0: feat_scaled = feat * dinv (bf16 table) ----
            with nc.named_scope("prescale"):
                for s in range(nstr):
                    rows = min(P, n - s * P)
                    fc = fscp.tile([P, IN_DIM], f32, name=f"fc_{s}", tag="fc")
                    nc.sync.dma_start(fc[:rows, :], feat_d[s * P : s * P + rows, :])
                    fs = fscp.tile([P, IN_DIM], bf16, name=f"fs_{s}", tag="fs")
                    nc.vector.tensor_scalar(
                        fs[:rows, :], fc[:rows, :], dinv_sb[:rows, s : s + 1],
                        None, ALU.mult,
                    )
                    nc.sync.dma_start(
                        feat_sc[s * P : s * P + rows, :], fs[:rows, :]
                    )

            def mp_layer(lname, src_dram, koff, klist, dl_sb, ix_sb, sc_col):
                """One aggregation layer -> list of per-tile SBUF tiles
                aggT [feat, dst] (bf16, already output-scaled)."""
                out_tiles = []
                for t in range(t_tiles):
                    kt = klist[t]
                    g = gp.tile([P, kmax, P], bf16, name=f"g_{lname}_{t}", tag="g")
                    for j in range(0, kt, GSPLIT):
                        jc = min(GSPLIT, kt - j)
                        o8 = 8 * (int(koff[t]) + j)
                        nc.gpsimd.dma_gather(
                            g[:, j : j + jc, :],
                            src_dram[:, :],
                            ix_sb[:, o8 : o8 + 8 * jc],
                            jc * P,
                            jc * P,
                            P,
                            queue_num=qc[0] % NQUEUES,
                        )
                        qc[0] += 1
                    ps = pag.tile([P, P], f32, name=f"agg_{lname}_{t}", tag="agg")
                    for k in range(kt):
                        c = int(koff[t]) + k
                        oh = ohp.tile([P, P], bf16, name=f"oh_{lname}_{t}_{k}", tag="oh")
                        nc.vector.tensor_scalar(
                            oh[:, :], iota_sb[:, :], dl_sb[:, c : c + 1],
                            None, ALU.is_equal,
                        )
                        nc.tensor.matmul(
                            ps[:, :], oh[:, :], g[:, k, :],
                            start=(k == 0), stop=(k == kt - 1),
                        )
                    # scale by per-dst coefficient while copying PSUM->SBUF
                    agg_nm = sbm.tile([P, P], bf16, name=f"anm_{lname}_{t}", tag="anm")
                    nc.vector.tensor_scalar(
                        agg_nm[:, :], ps[:, :], sc_col[:, t : t + 1], None, ALU.mult
                    )
                    # transpose to [feat, dst]
                    pt = pst.tile([P, P], bf16, name=f"pt_{lname}_{t}", tag="pt")
                    nc.tensor.transpose(pt[:, :], agg_nm[:, :], ident_sb[:, :])
                    aggT = sbm.tile([P, P], bf16, name=f"aggT_{lname}_{t}", tag="aggT")
                    nc.scalar.activation(aggT[:, :], pt[:, :], AF.Copy)
                    out_tiles.append(aggT)
                return out_tiles

            # ======== Layer 1: GCN ========
            with nc.named_scope("layer1"):
                agg1 = mp_layer("l1", feat_sc, k1off, k1, dl1_sb, ix1_sb, sc1_sb)
                for t in range(t_tiles):
                    ph = psh.tile([P, P], f32, name=f"ph_{t}", tag="ps_h")
                    nc.tensor.matmul(
                        ph[:, :], wg_sb[:, :], agg1[t][:, :], start=True, stop=True
                    )
                    nc.scalar.activation(
                        h_sb[:, t * P : (t + 1) * P], ph[:, :], AF.Relu, bias=b_gcn
                    )
                    pt2 = pst.tile([P, P], bf16, name=f"pt2_{t}", tag="pt")
                    nc.tensor.transpose(
                        pt2[:, :], h_sb[:, t * P : (t + 1) * P], ident_sb[:, :]
                    )
                    hnm = sbm.tile([P, P], bf16, name=f"hnm_{t}", tag="hnm")
                    nc.vector.tensor_copy(hnm[:, :], pt2[:, :])
                    rows = min(P, npn - t * P)
                    nc.sync.dma_start(
                        h_own[t * P : t * P + rows, :], hnm[:rows, :]
                    )

            # ======== AllGather h ========
            with nc.named_scope("allgather"):
                nc.gpsimd.collective_compute(
                    "AllGather",
                    mybir.AluOpType.bypass,
                    replica_groups=[list(range(ncores))],
                    ins=[h_own.opt()],
                    outs=[h_full.opt()],
                )

            # ======== Layer 2: SAGE(mean) ========
            with nc.named_scope("layer2"):
                agg2 = mp_layer("l2", h_full, k2off, k2, dl2_sb, ix2_sb, sc2_sb)
                for t in range(t_tiles):
                    pz = psh.tile([P, P], f32, name=f"pz_{t}", tag="ps_h")
                    nc.tensor.matmul(
                        pz[:, :], wsl_sb[:, :], agg2[t][:, :], start=True, stop=False
                    )
                    nc.tensor.matmul(
                        pz[:, :], wsr_sb[:, :], h_sb[:, t * P : (t + 1) * P],
                        start=False, stop=True,
                    )
                    nc.scalar.activation(
                        h2_sb[:, t * P : (t + 1) * P], pz[:, :], AF.Relu, bias=b_sage
                    )

            # ======== Policy MLP + value head (512-column groups) ========
            with nc.named_scope("mlp"):
                for g_i in range(ngrp):
                    lo = g_i * 512
                    hi = min(nsb, lo + 512)
                    w = hi - lo
                    p1 = pml.tile([P, 512], f32, name=f"p1_{g_i}", tag="pmlp")[:, :w]
                    nc.tensor.matmul(
                        p1, w1_sb[:, :], h2_sb[:, lo:hi], start=True, stop=True
                    )
                    z1 = mlpp.tile([P, 512], bf16, name=f"z1_{g_i}", tag="z")[:, :w]
                    nc.scalar.activation(z1, p1, gelu_af, bias=b1c)
                    p2 = pml.tile([P, 512], f32, name=f"p2_{g_i}", tag="pmlp")[:, :w]
                    nc.tensor.matmul(p2, w2_sb[:, :], z1, start=True, stop=True)
                    z2 = mlpp.tile([P, 512], bf16, name=f"z2_{g_i}", tag="z")[:, :w]
                    nc.scalar.activation(z2, p2, gelu_af, bias=b2c)
                    p3 = pou.tile([OUT, 512], f32, name=f"p3_{g_i}", tag="pout")[:, :w]
                    nc.tensor.matmul(p3, w3_sb[:, :], z2, start=True, stop=True)
                    nc.vector.tensor_scalar_add(means_sb[:, lo:hi], p3, b3c[:OUT, :])
                    pv = pou.tile([1, 512], f32, name=f"pv_{g_i}", tag="pv")[:, :w]
                    nc.tensor.matmul(
                        pv, wv_sb[:, :], h2_sb[:, lo:hi], start=True, stop=True
                    )
                    nc.vector.tensor_scalar_add(vals_sb[:, lo:hi], pv, bvc[:1, :])

            # ======== outputs ========
            nc.sync.dma_start(means_d[:, :], means_sb[:, :npn])
            nc.sync.dma_start(vals_d[:, :], vals_sb[:, :npn])

    nc.compile()
    return nc


# --------------------------------------------------------------------------
# In-map assembly + entry point
# --------------------------------------------------------------------------

def make_in_maps(inputs, prep, n, ncores):
    import ml_dtypes

    bf = ml_dtypes.bfloat16
    pk1, pk2, dinv_all, sc1, sc2 = prep
    ix1, dl1, _ = pk1
    ix2, dl2, _ = pk2
    f = np.ascontiguousarray(np.asarray(inputs["feat"], np.float32))
    biases = np.zeros((P, 8), np.float32)
    for col, key in enumerate(["b_gcn", "b_sage_l", "b1", "b2", "b3", "bv"]):
        b = np.asarray(inputs[key], np.float32).reshape(-1)
        biases[: b.shape[0], col] = b
    iota = np.tile(np.arange(P, dtype=np.float32), (P, 1)).astype(bf)
    ident = np.eye(P, dtype=np.float32).astype(bf)
    common = {
        "feat": f,
        "W_gcn": np.asarray(inputs["W_gcn"], np.float32).astype(bf),
        "W_sage_l": np.asarray(inputs["W_sage_l"], np.float32).astype(bf),
        "W_sage_r": np.asarray(inputs["W_sage_r"], np.float32).astype(bf),
        "W1": np.asarray(inputs["W1"], np.float32).astype(bf),
        "W2": np.asarray(inputs["W2"], np.float32).astype(bf),
        "W3": np.asarray(inputs["W3"], np.float32).astype(bf),
        "Wv": np.asarray(inputs["Wv"], np.float32).astype(bf),
        "biases": biases,
        "iota": np.ascontiguousarray(iota),
        "ident": np.ascontiguousarray(ident),
        "dinv_all": np.ascontiguousarray(dinv_all),
    }
    in_maps = []
    for c in range(ncores):
        m = dict(common)
        m["idxs1"] = np.ascontiguousarray(ix1[c])
        m["dl1"] = np.ascontiguousarray(dl1[c])
        m["idxs2"] = np.ascontiguousarray(ix2[c])
        m["dl2"] = np.ascontiguousarray(dl2[c])
        m["sc1"] = np.ascontiguousarray(sc1[c])
        m["sc2"] = np.ascontiguousarray(sc2[c])
        in_maps.append(m)
    return in_maps


_CACHE = {}


def _get_program(n, ncores, k1, k2):
    key = (n, ncores, tuple(k1), tuple(k2))
    if key not in _CACHE:
        _CACHE[key] = build_program(n, ncores, k1, k2)
    return _CACHE[key]


def _ensure_ntff_hook():
    """This image's ``antenv`` lacks ``axon_hooks``; register a shim module
    and wire the ctypes NTFF profile hook from the boot helper so
    ``run_bass_kernel_spmd(trace=True)`` can capture profiles."""
    import sys
    import types

    try:
        import antenv.axon_hooks  # noqa: F401

        return
    except ImportError:
        pass
    import antenv

    mod = types.ModuleType("antenv.axon_hooks")
    state = {"hook": None}
    mod.set_axon_ntff_profile_hook = lambda h: state.__setitem__("hook", h)
    mod.get_axon_ntff_profile_hook = lambda: state["hook"]
    sys.modules["antenv.axon_hooks"] = mod
    antenv.axon_hooks = mod
    try:
        from trn_agent_boot.trn_boot import _ntff_profile_via_ctypes

        import os

        so = "/opt/axon/libaxon_pjrt.so"
        if os.path.exists(so):
            state["hook"] = _ntff_profile_via_ctypes(so)
    except Exception:
        pass


def run(inputs, n=N, ncores=NCORES, trace=False):
    """Full pipeline: prep, build/compile (cached), execute, unshard."""
    from concourse.bass_utils import run_bass_kernel_spmd

    if trace:
        _ensure_ntff_hook()
    prep = prepare(np.asarray(inputs["edge_index"]), n, ncores)
    nc = _get_program(n, ncores, prep[0][2], prep[1][2])
    in_maps = make_in_maps(inputs, prep, n, ncores)
    res = run_bass_kernel_spmd(
        nc, in_maps, core_ids=list(range(ncores)), trace=trace
    )
    npn = n // ncores
    means = np.empty((n, OUT), np.float32)
    values = np.empty((n,), np.float32)
    for c in range(ncores):
        means[c * npn : (c + 1) * npn, :] = res.results[c]["meansT_out"].T
        values[c * npn : (c + 1) * npn] = res.results[c]["vals_out"][0]
    return (means, values), res


def kernel(**inputs):
    out, _ = run(inputs)
    return out


# revision 17
# speedup vs baseline: 2.0611x; 1.2182x over previous
"""GNN message-passing kernel for Trainium2 (8 NeuronCores, Bass/Tile).

Problem: GCNConv -> ReLU -> SAGEConv(mean) -> ReLU -> policy MLP (GELU x2) +
value head, on a 20000-node / 640000-edge random graph, 128-dim features.

Strategy (per sharding hint): shard nodes/output rows across the 8 cores;
partition edges by destination node.  Host does *index-only* preparation
(edge bucketing by destination tile, int16 gather indices, normalization
coefficient tables).  Device pipeline per core:

  phase 0   feat_scaled[n] = feat[n] * dinv[n]  (bf16 table in DRAM)
  layer 1   per dst tile: dma_gather feat_scaled rows for the tile's edges
            (1024-idx pieces round-robined over 4 SWDGE queues -- the
            descriptor ring is tiny and a single queue serializes), build
            0/1 one-hot selection matrices on VectorE (iota == dst_local,
            one bf16 tensor_scalar per 128-edge chunk), segment-sum via
            TensorE matmul accumulation: agg[d,f] += onehot[e,d].T @ g[e,f].
            Scale by dinv[d] (per-partition) while copying PSUM->SBUF, PE-
            transpose to [f,d], then h = relu(W_gcn.T @ aggT + b).
            GCN self-loops ride along as ordinary edges: gathered value is
            dinv[d]*feat[d] and the output-side dinv[d] completes dinv^2.
  allgather h (node-major bf16) so layer 2 can gather any source row.
  layer 2   same aggregation over h with scale 1/max(cnt[d],1) (SAGE mean),
            then h2 = relu(W_l.T @ meanT + b_l + W_r.T @ hT).
  mlp       z=gelu(W1.T z+b1) x2, means=W3.T z+b3, values=Wv.T h2+bv in
            512-column batches.  Outputs stored transposed; host re-stacks.

Aggregation runs in bf16 (fp32 PSUM accumulate); rounding errors are
independent per edge so the segment means stay ~1e-3 accurate.
"""

import math

import numpy as np

# Problem constants (hardcoded per harness contract).
N = 20000
E = 640000
IN_DIM = 128
HID = 128
OUT = 64
NCORES = 8
P = 128  # partitions / tile width
GSPLIT = 8  # chunks per dma_gather: 1024 idxs = 65 descs/lane (ring limit ~80)
NQUEUES = 4


# --------------------------------------------------------------------------
# Host-side preparation (index bookkeeping only -- all FLOPs on device)
# --------------------------------------------------------------------------

def _pack_layer(s, d, n, ncores):
    """Bucket edges by (core, dst-tile), pad each tile's edge list to a
    multiple of 128, and emit per-core device arrays:

      idxs [ncores, 128, 8*sum(K)] int16  -- dma_gather indices (16-wrapped,
                                             replicated to 128 partitions)
      dl   [ncores, 128, sum(K)]  f32     -- dst-local index (0..127) / edge
      K    [T] int                         -- chunks (of 128 edges) per tile

    Edge slot i of a tile lands at partition i%128, chunk i//128 (dma_gather
    output layout).  Pad slots: src=0, dl=-1 (one-hot row all zero).
    """
    npn = n // ncores
    t_tiles = math.ceil(npn / P)
    core = d // npn
    dloc = d % npn
    tile_id = dloc // P
    dl_val = (dloc % P).astype(np.float32)
    bucket = core * t_tiles + tile_id
    order = np.argsort(bucket, kind="stable")
    s_s = s[order]
    dl_s = dl_val[order]
    counts = np.bincount(bucket, minlength=ncores * t_tiles).reshape(
        ncores, t_tiles
    )
    bounds = np.zeros(ncores * t_tiles + 1, np.int64)
    np.cumsum(counts.reshape(-1), out=bounds[1:])
    kk = np.maximum((counts.max(axis=0) + P - 1) // P, 1).astype(np.int64)
    ksum = int(kk.sum())
    idxs = np.zeros((ncores, P, 8 * ksum), np.int16)
    dl = np.full((ncores, P, ksum), -1.0, np.float32)
    koff = np.zeros(t_tiles + 1, np.int64)
    np.cumsum(kk, out=koff[1:])
    for c in range(ncores):
        for t in range(t_tiles):
            b = c * t_tiles + t
            lo, hi = bounds[b], bounds[b + 1]
            m = hi - lo
            slots = int(kk[t]) * P
            sp = np.zeros(slots, np.int16)
            sp[:m] = s_s[lo:hi].astype(np.int16)
            dp = np.full(slots, -1.0, np.float32)
            dp[:m] = dl_s[lo:hi]
            o8 = 8 * int(koff[t])
            idxs[c, :, o8 : o8 + slots // 16] = np.tile(
                sp.reshape(slots // 16, 16).T, (8, 1)
            )
            ok = int(koff[t])
            dl[c, :, ok : ok + int(kk[t])] = dp.reshape(int(kk[t]), P).T
    return idxs, dl, [int(x) for x in kk]


def prepare(edge_index, n, ncores):
    """All host-side index prep for both message-passing layers."""
    src = np.asarray(edge_index[0]).astype(np.int64)
    dst = np.asarray(edge_index[1]).astype(np.int64)
    npn = n // ncores
    t_tiles = math.ceil(npn / P)
    indeg = np.bincount(dst, minlength=n).astype(np.float64)
    dinv = (1.0 / np.sqrt(indeg + 1.0)).astype(np.float32)  # GCN w/ self-loop
    cntinv = (1.0 / np.maximum(indeg, 1.0)).astype(np.float32)
    loop = np.arange(n, dtype=np.int64)
    s1 = np.concatenate([src, loop])
    d1 = np.concatenate([dst, loop])
    pk1 = _pack_layer(s1, d1, n, ncores)
    pk2 = _pack_layer(src, dst, n, ncores)
    # per-node tables, wrapped [128, n/128]: node m at [m%128, m//128]
    nstr = math.ceil(n / P)
    dpad = np.concatenate([dinv, np.zeros(nstr * P - n, np.float32)])
    dinv_all = np.ascontiguousarray(dpad.reshape(nstr, P).T)
    # per-core output-side scales per own tile: [128, T]
    sc1 = np.zeros((ncores, P, t_tiles), np.float32)
    sc2 = np.zeros((ncores, P, t_tiles), np.float32)
    for c in range(ncores):
        own = np.arange(c * npn, (c + 1) * npn)
        pad = t_tiles * P - npn
        d1v = np.concatenate([dinv[own], np.zeros(pad, np.float32)])
        d2v = np.concatenate([cntinv[own], np.zeros(pad, np.float32)])
        sc1[c] = d1v.reshape(t_tiles, P).T
        sc2[c] = d2v.reshape(t_tiles, P).T
    return pk1, pk2, dinv_all, sc1, sc2


# --------------------------------------------------------------------------
# Device program
# --------------------------------------------------------------------------

def build_program(n, ncores, k1, k2, mlp_af=None):
    """Build the SPMD Bass/Tile program (identical on all cores; per-core
    behavior comes from per-core input data).  mlp_af overrides the MLP
    activation (CoreSim lacks Gelu; tests pass Tanh)."""
    import concourse.bass as bass  # noqa: F401
    import concourse.mybir as mybir
    import concourse.tile as tile
    from concourse import bacc

    f32 = mybir.dt.float32
    bf16 = mybir.dt.bfloat16
    i16 = mybir.dt.int16
    AF = mybir.ActivationFunctionType
    ALU = mybir.AluOpType
    gelu_af = AF.Gelu if mlp_af is None else mlp_af

    npn = n // ncores
    t_tiles = len(k1)
    kmax = max(max(k1), max(k2))
    nstr = math.ceil(n / P)
    k1off = np.zeros(t_tiles + 1, np.int64)
    np.cumsum(k1, out=k1off[1:])
    k2off = np.zeros(t_tiles + 1, np.int64)
    np.cumsum(k2, out=k2off[1:])
    nsb = t_tiles * P  # padded node count per core in SBUF (2560)
    ngrp = math.ceil(nsb / 512)  # MLP groups of 512 columns

    nc = bacc.Bacc(
        "TRN2",
        target_bir_lowering=False,
        debug=False,
        num_devices=ncores,
        num_swdge_queues=NQUEUES,
    )
    qc = [0]  # SWDGE queue rotation counter

    # ---- I/O ----
    feat_d = nc.dram_tensor("feat", [n, IN_DIM], f32, kind="ExternalInput")
    w_gcn_d = nc.dram_tensor("W_gcn", [IN_DIM, HID], bf16, kind="ExternalInput")
    w_sl_d = nc.dram_tensor("W_sage_l", [HID, HID], bf16, kind="ExternalInput")
    w_sr_d = nc.dram_tensor("W_sage_r", [HID, HID], bf16, kind="ExternalInput")
    w1_d = nc.dram_tensor("W1", [HID, HID], bf16, kind="ExternalInput")
    w2_d = nc.dram_tensor("W2", [HID, HID], bf16, kind="ExternalInput")
    w3_d = nc.dram_tensor("W3", [HID, OUT], bf16, kind="ExternalInput")
    wv_d = nc.dram_tensor("Wv", [HID, 1], bf16, kind="ExternalInput")
    bias_d = nc.dram_tensor("biases", [P, 8], f32, kind="ExternalInput")
    iota_d = nc.dram_tensor("iota", [P, P], bf16, kind="ExternalInput")
    ident_d = nc.dram_tensor("ident", [P, P], bf16, kind="ExternalInput")
    dinv_d = nc.dram_tensor("dinv_all", [P, nstr], f32, kind="ExternalInput")
    sc1_d = nc.dram_tensor("sc1", [P, t_tiles], f32, kind="ExternalInput")
    sc2_d = nc.dram_tensor("sc2", [P, t_tiles], f32, kind="ExternalInput")
    ix1_d = nc.dram_tensor("idxs1", [P, 8 * k1off[-1]], i16, kind="ExternalInput")
    dl1_d = nc.dram_tensor("dl1", [P, int(k1off[-1])], f32, kind="ExternalInput")
    ix2_d = nc.dram_tensor("idxs2", [P, 8 * k2off[-1]], i16, kind="ExternalInput")
    dl2_d = nc.dram_tensor("dl2", [P, int(k2off[-1])], f32, kind="ExternalInput")
    means_d = nc.dram_tensor("meansT_out", [OUT, npn], f32, kind="ExternalOutput")
    vals_d = nc.dram_tensor("vals_out", [1, npn], f32, kind="ExternalOutput")

    with tile.TileContext(nc) as tc:
        with (
            tc.tile_pool(name="const", bufs=1) as cp,
            tc.tile_pool(name="fsc", bufs=4) as fscp,
            tc.tile_pool(name="gath", bufs=3) as gp,
            tc.tile_pool(name="oh", bufs=8) as ohp,
            tc.tile_pool(name="sbm", bufs=3) as sbm,
            tc.tile_pool(name="mlp", bufs=2) as mlpp,
            tc.tile_pool(name="ps_agg", bufs=2, space="PSUM") as pag,
            tc.tile_pool(name="ps_t", bufs=2, space="PSUM") as pst,
            tc.tile_pool(name="ps_h", bufs=1, space="PSUM") as psh,
            tc.tile_pool(name="ps_mlp", bufs=1, space="PSUM") as pml,
            tc.tile_pool(name="ps_out", bufs=1, space="PSUM") as pou,
            tc.tile_pool(name="dram", bufs=1, space="DRAM") as dp,
        ):
            # ---- constants into SBUF ----
            wg_sb = cp.tile_from(w_gcn_d[:, :])
            wsl_sb = cp.tile_from(w_sl_d[:, :])
            wsr_sb = cp.tile_from(w_sr_d[:, :])
            w1_sb = cp.tile_from(w1_d[:, :])
            w2_sb = cp.tile_from(w2_d[:, :])
            w3_sb = cp.tile_from(w3_d[:, :])
            wv_sb = cp.tile_from(wv_d[:, :])
            bias_sb = cp.tile_from(bias_d[:, :])
            iota_sb = cp.tile_from(iota_d[:, :])
            ident_sb = cp.tile_from(ident_d[:, :])
            dinv_sb = cp.tile_from(dinv_d[:, :])
            sc1_sb = cp.tile_from(sc1_d[:, :])
            sc2_sb = cp.tile_from(sc2_d[:, :])
            ix1_sb = cp.tile_from(ix1_d[:, :])
            dl1_sb = cp.tile_from(dl1_d[:, :])
            ix2_sb = cp.tile_from(ix2_d[:, :])
            dl2_sb = cp.tile_from(dl2_d[:, :])
            b_gcn = bias_sb[:, 0:1]
            b_sage = bias_sb[:, 1:2]
            b1c = bias_sb[:, 2:3]
            b2c = bias_sb[:, 3:4]
            b3c = bias_sb[:, 4:5]
            bvc = bias_sb[:, 5:6]

            h_sb = cp.tile([P, nsb], bf16)    # layer-1 out, [hid, node]
            h2_sb = cp.tile([P, nsb], bf16)   # layer-2 out
            means_sb = cp.tile([OUT, nsb], f32)
            vals_sb = cp.tile([1, nsb], f32)

            feat_sc = dp.tile([nstr * P, IN_DIM], bf16)
            h_own = dp.tile([npn, HID], bf16)
            h_full = dp.tile([n, HID], bf16, addr_space="Shared")

            # ---- phase 0: feat_scaled = feat * dinv (bf16 table) ----
            # Loads batched 4 stripes per DMA; per-stripe scale on ScalarE
            # (activation Copy with per-partition scale AP) keeps DVE free.
            with nc.named_scope("prescale"):
                nfull = n // P  # full 128-row stripes; tail handled solo
                bs = 4
                for s0 in range(0, nfull, bs):
                    nb = min(bs, nfull - s0)
                    fc = fscp.tile([P, bs, IN_DIM], f32, name=f"fc_{s0}", tag="fc")
                    nc.sync.dma_start(
                        fc[:, :nb, :],
                        feat_d[s0 * P : (s0 + nb) * P, :].rearrange(
                            "(a p) f -> p a f", p=P
                        ),
                    )
                    fs = fscp.tile([P, bs, IN_DIM], bf16, name=f"fs_{s0}", tag="fs")
                    for a in range(nb):
                        nc.scalar.activation(
                            fs[:, a, :], fc[:, a, :], AF.Copy,
                            scale=dinv_sb[:, s0 + a : s0 + a + 1],
                        )
                    nc.sync.dma_start(
                        feat_sc[s0 * P : (s0 + nb) * P, :].rearrange(
                            "(a p) f -> p a f", p=P
                        ),
                        fs[:, :nb, :],
                    )
                if n % P:
                    s0 = nfull
                    rows = n - nfull * P
                    fcl = fscp.tile([P, IN_DIM], f32, name="fc_last", tag="fcl")
                    nc.sync.dma_start(fcl[:rows, :], feat_d[s0 * P :, :])
                    fsl = fscp.tile([P, IN_DIM], bf16, name="fs_last", tag="fsl")
                    nc.vector.memset(fsl[:, :], 0.0)
                    nc.scalar.activation(
                        fsl[:rows, :], fcl[:rows, :], AF.Copy,
                        scale=dinv_sb[:rows, s0 : s0 + 1],
                    )
                    nc.sync.dma_start(
                        feat_sc[s0 * P : (s0 + 1) * P, :], fsl[:, :]
                    )

            def mp_layer(lname, src_dram, koff, klist, dl_sb, ix_sb, sc_col):
                """One aggregation layer -> list of per-tile SBUF tiles
                aggT [feat, dst] (bf16, already output-scaled)."""
                out_tiles = []
                for t in range(t_tiles):
                    kt = klist[t]
                    g = gp.tile([P, kmax, P], bf16, name=f"g_{lname}_{t}", tag="g")
                    for j in range(0, kt, GSPLIT):
                        jc = min(GSPLIT, kt - j)
                        o8 = 8 * (int(koff[t]) + j)
                        nc.gpsimd.dma_gather(
                            g[:, j : j + jc, :],
                            src_dram[:, :],
                            ix_sb[:, o8 : o8 + 8 * jc],
                            jc * P,
                            jc * P,
                            P,
                            queue_num=qc[0] % NQUEUES,
                        )
                        qc[0] += 1
                    ps = pag.tile([P, P], f32, name=f"agg_{lname}_{t}", tag="agg")
                    for k in range(kt):
                        c = int(koff[t]) + k
                        oh = ohp.tile([P, P], bf16, name=f"oh_{lname}_{t}_{k}", tag="oh")
                        nc.vector.tensor_scalar(
                            oh[:, :], iota_sb[:, :], dl_sb[:, c : c + 1],
                            None, ALU.is_equal,
                        )
                        nc.tensor.matmul(
                            ps[:, :], oh[:, :], g[:, k, :],
                            start=(k == 0), stop=(k == kt - 1),
                        )
                    # scale by per-dst coefficient while copying PSUM->SBUF
                    agg_nm = sbm.tile([P, P], bf16, name=f"anm_{lname}_{t}", tag="anm")
                    nc.scalar.activation(
                        agg_nm[:, :], ps[:, :], AF.Copy, scale=sc_col[:, t : t + 1]
                    )
                    # transpose to [feat, dst]
                    pt = pst.tile([P, P], bf16, name=f"pt_{lname}_{t}", tag="pt")
                    nc.tensor.transpose(pt[:, :], agg_nm[:, :], ident_sb[:, :])
                    aggT = sbm.tile([P, P], bf16, name=f"aggT_{lname}_{t}", tag="aggT")
                    nc.scalar.activation(aggT[:, :], pt[:, :], AF.Copy)
                    out_tiles.append(aggT)
                return out_tiles

            # ======== Layer 1: GCN ========
            with nc.named_scope("layer1"):
                agg1 = mp_layer("l1", feat_sc, k1off, k1, dl1_sb, ix1_sb, sc1_sb)
                for t in range(t_tiles):
                    ph = psh.tile([P, P], f32, name=f"ph_{t}", tag="ps_h")
                    nc.tensor.matmul(
                        ph[:, :], wg_sb[:, :], agg1[t][:, :], start=True, stop=True
                    )
                    nc.scalar.activation(
                        h_sb[:, t * P : (t + 1) * P], ph[:, :], AF.Relu, bias=b_gcn
                    )
                    pt2 = pst.tile([P, P], bf16, name=f"pt2_{t}", tag="pt")
                    nc.tensor.transpose(
                        pt2[:, :], h_sb[:, t * P : (t + 1) * P], ident_sb[:, :]
                    )
                    hnm = sbm.tile([P, P], bf16, name=f"hnm_{t}", tag="hnm")
                    nc.scalar.activation(hnm[:, :], pt2[:, :], AF.Copy)
                    rows = min(P, npn - t * P)
                    nc.sync.dma_start(
                        h_own[t * P : t * P + rows, :], hnm[:rows, :]
                    )

            # ======== AllGather h ========
            with nc.named_scope("allgather"):
                nc.gpsimd.collective_compute(
                    "AllGather",
                    mybir.AluOpType.bypass,
                    replica_groups=[list(range(ncores))],
                    ins=[h_own.opt()],
                    outs=[h_full.opt()],
                )

            # ======== Layer 2: SAGE(mean) ========
            with nc.named_scope("layer2"):
                agg2 = mp_layer("l2", h_full, k2off, k2, dl2_sb, ix2_sb, sc2_sb)
                for t in range(t_tiles):
                    pz = psh.tile([P, P], f32, name=f"pz_{t}", tag="ps_h")
                    nc.tensor.matmul(
                        pz[:, :], wsl_sb[:, :], agg2[t][:, :], start=True, stop=False
                    )
                    nc.tensor.matmul(
                        pz[:, :], wsr_sb[:, :], h_sb[:, t * P : (t + 1) * P],
                        start=False, stop=True,
                    )
                    nc.scalar.activation(
                        h2_sb[:, t * P : (t + 1) * P], pz[:, :], AF.Relu, bias=b_sage
                    )

            # ======== Policy MLP + value head (512-column groups) ========
            with nc.named_scope("mlp"):
                for g_i in range(ngrp):
                    lo = g_i * 512
                    hi = min(nsb, lo + 512)
                    w = hi - lo
                    p1 = pml.tile([P, 512], f32, name=f"p1_{g_i}", tag="pmlp")[:, :w]
                    nc.tensor.matmul(
                        p1, w1_sb[:, :], h2_sb[:, lo:hi], start=True, stop=True
                    )
                    z1 = mlpp.tile([P, 512], bf16, name=f"z1_{g_i}", tag="z")[:, :w]
                    nc.scalar.activation(z1, p1, gelu_af, bias=b1c)
                    p2 = pml.tile([P, 512], f32, name=f"p2_{g_i}", tag="pmlp")[:, :w]
                    nc.tensor.matmul(p2, w2_sb[:, :], z1, start=True, stop=True)
                    z2 = mlpp.tile([P, 512], bf16, name=f"z2_{g_i}", tag="z")[:, :w]
                    nc.scalar.activation(z2, p2, gelu_af, bias=b2c)
                    p3 = pou.tile([OUT, 512], f32, name=f"p3_{g_i}", tag="pout")[:, :w]
                    nc.tensor.matmul(p3, w3_sb[:, :], z2, start=True, stop=True)
                    nc.scalar.activation(means_sb[:, lo:hi], p3, AF.Identity, bias=b3c[:OUT, :])
                    pv = pou.tile([1, 512], f32, name=f"pv_{g_i}", tag="pv")[:, :w]
                    nc.tensor.matmul(
                        pv, wv_sb[:, :], h2_sb[:, lo:hi], start=True, stop=True
                    )
                    nc.scalar.activation(vals_sb[:, lo:hi], pv, AF.Identity, bias=bvc[:1, :])

            # ======== outputs ========
            nc.sync.dma_start(means_d[:, :], means_sb[:, :npn])
            nc.sync.dma_start(vals_d[:, :], vals_sb[:, :npn])

    nc.compile()
    return nc


# --------------------------------------------------------------------------
# In-map assembly + entry point
# --------------------------------------------------------------------------

def make_in_maps(inputs, prep, n, ncores):
    import ml_dtypes

    bf = ml_dtypes.bfloat16
    pk1, pk2, dinv_all, sc1, sc2 = prep
    ix1, dl1, _ = pk1
    ix2, dl2, _ = pk2
    f = np.ascontiguousarray(np.asarray(inputs["feat"], np.float32))
    biases = np.zeros((P, 8), np.float32)
    for col, key in enumerate(["b_gcn", "b_sage_l", "b1", "b2", "b3", "bv"]):
        b = np.asarray(inputs[key], np.float32).reshape(-1)
        biases[: b.shape[0], col] = b
    iota = np.tile(np.arange(P, dtype=np.float32), (P, 1)).astype(bf)
    ident = np.eye(P, dtype=np.float32).astype(bf)
    common = {
        "feat": f,
        "W_gcn": np.asarray(inputs["W_gcn"], np.float32).astype(bf),
        "W_sage_l": np.asarray(inputs["W_sage_l"], np.float32).astype(bf),
        "W_sage_r": np.asarray(inputs["W_sage_r"], np.float32).astype(bf),
        "W1": np.asarray(inputs["W1"], np.float32).astype(bf),
        "W2": np.asarray(inputs["W2"], np.float32).astype(bf),
        "W3": np.asarray(inputs["W3"], np.float32).astype(bf),
        "Wv": np.asarray(inputs["Wv"], np.float32).astype(bf),
        "biases": biases,
        "iota": np.ascontiguousarray(iota),
        "ident": np.ascontiguousarray(ident),
        "dinv_all": np.ascontiguousarray(dinv_all),
    }
    in_maps = []
    for c in range(ncores):
        m = dict(common)
        m["idxs1"] = np.ascontiguousarray(ix1[c])
        m["dl1"] = np.ascontiguousarray(dl1[c])
        m["idxs2"] = np.ascontiguousarray(ix2[c])
        m["dl2"] = np.ascontiguousarray(dl2[c])
        m["sc1"] = np.ascontiguousarray(sc1[c])
        m["sc2"] = np.ascontiguousarray(sc2[c])
        in_maps.append(m)
    return in_maps


_CACHE = {}


def _get_program(n, ncores, k1, k2):
    key = (n, ncores, tuple(k1), tuple(k2))
    if key not in _CACHE:
        _CACHE[key] = build_program(n, ncores, k1, k2)
    return _CACHE[key]


def _ensure_ntff_hook():
    """This image's ``antenv`` lacks ``axon_hooks``; register a shim module
    and wire the ctypes NTFF profile hook from the boot helper so
    ``run_bass_kernel_spmd(trace=True)`` can capture profiles."""
    import sys
    import types

    try:
        import antenv.axon_hooks  # noqa: F401

        return
    except ImportError:
        pass
    import antenv

    mod = types.ModuleType("antenv.axon_hooks")
    state = {"hook": None}
    mod.set_axon_ntff_profile_hook = lambda h: state.__setitem__("hook", h)
    mod.get_axon_ntff_profile_hook = lambda: state["hook"]
    sys.modules["antenv.axon_hooks"] = mod
    antenv.axon_hooks = mod
    try:
        from trn_agent_boot.trn_boot import _ntff_profile_via_ctypes

        import os

        so = "/opt/axon/libaxon_pjrt.so"
        if os.path.exists(so):
            state["hook"] = _ntff_profile_via_ctypes(so)
    except Exception:
        pass


def run(inputs, n=N, ncores=NCORES, trace=False):
    """Full pipeline: prep, build/compile (cached), execute, unshard."""
    from concourse.bass_utils import run_bass_kernel_spmd

    if trace:
        _ensure_ntff_hook()
    prep = prepare(np.asarray(inputs["edge_index"]), n, ncores)
    nc = _get_program(n, ncores, prep[0][2], prep[1][2])
    in_maps = make_in_maps(inputs, prep, n, ncores)
    res = run_bass_kernel_spmd(
        nc, in_maps, core_ids=list(range(ncores)), trace=trace
    )
    npn = n // ncores
    means = np.empty((n, OUT), np.float32)
    values = np.empty((n,), np.float32)
    for c in range(ncores):
        means[c * npn : (c + 1) * npn, :] = res.results[c]["meansT_out"].T
        values[c * npn : (c + 1) * npn] = res.results[c]["vals_out"][0]
    return (means, values), res


def kernel(**inputs):
    out, _ = run(inputs)
    return out
